# revision 17
# baseline (speedup 1.0000x reference)
"""Trainium2 Bass kernel for nn_DAMSoftmax (sub-center ArcFace loss, model-parallel softmax CE).

Contract: kernel(**inputs) takes FULL inputs {input:(1024,128) f32, factor:(1024,1) f32,
label:(1024,) int, weight:(16,128,10000) f32} and returns (cls_loss, prec1) scalars,
matching the reference.

Strategy (v8, class-partition orientation):
  - Shard OUT=10000 classes across 8 cores (1250 each, zero-padded to 1280 =
    10 chunks x 128 classes).
  - Host: L2-normalize input rows / weight columns; upload fp16 xnT (128,1024)
    [the moving operand of every matmul] + per-core fp16 weight shard
    (128, 16*1280) [stationary (128,128) blocks].
  - Device (per core), per class-chunk c (10 chunks), per sub-center k (16):
    matmul -> PSUM plane (128 classes, 1024 batch) fp32 = exactly 2 banks, so
    the PSUM ring is 3 deep (plus 2 banks for zsum). Planes drain via
    11 ACT evictions + 5 DVE direct maxes per chunk; DVE merges evicted
    fp16 tiles into the chunk acc (max over k).
  - Per chunk: ACT Exp(S*x) on the acc -> bf16 escratch (range e^+-29 fits
    easily; bf16 keeps DVE-free); PE "ones" matmul reduces escratch over the
    128 class-partitions, accumulating into zsum (1,1024) PSUM fp32 across
    all 10 chunks. exp and ones-matmuls are deferred 1-2 chunks so they
    never block the ACT / PE queue heads.
  - Device output per core: (1, 1024) fp32 = per-row sum of exp(S*cos) over
    the core's classes. Host: exact cross-core sum, label-column
    replacement (margined logit recomputed exactly), prec1 via bound
    screening with exact fallback for ambiguous rows.
"""

import math
import numpy as np

S = 64.0
MARGIN = 0.5
C = 1.5
K = 16
EPS = 1e-6
IN = 128
OUT = 10000
B = 1024
NCORES = 8
OSH = OUT // NCORES        # 1250 real classes per core
NCHUNK = 10                # class chunks of 128 (1280 padded)
OSHP = NCHUNK * 128        # 1280 padded classes per core

# Per-chunk plane-drain schedule: "a" = ACT eviction (first writes acc),
# "d" = DVE direct max from PSUM. Alternating a=11 and a=10 chunks balances
# ACT against DVE (5-6 directs + 9-10 fp16 merges). Leading "a"s let a new
# chunk's drain proceed on ACT while DVE clears the previous chunk's
# merge backlog.
SCHED11 = "aaadaadaadaadaad"  # a=11, d=5
SCHED10 = "aadaadaadaadadad"  # a=10, d=6


def _build_nc_v8():
    import concourse.bacc as bacc
    import concourse.tile as tile
    from concourse import mybir

    f32 = mybir.dt.float32
    f16 = mybir.dt.float16
    bf16 = mybir.dt.bfloat16

    nc = bacc.Bacc(
        "TRN2", target_bir_lowering=False, debug=False, num_devices=NCORES
    )
    xnT_d = nc.declare_dram_parameter("xnT", (IN, B), f16, isOutput=False)
    w_d = nc.declare_dram_parameter("w", (IN, K * OSHP), f16, isOutput=False)
    out_d = nc.declare_dram_parameter("out", (1, B), f32, isOutput=True)

    with tile.TileContext(nc) as tc:
        with (
            tc.tile_pool(name="consts", bufs=1) as cpool,
            tc.tile_pool(name="wpool", bufs=1) as wpool,
            tc.tile_pool(name="psum", bufs=3, space="PSUM") as ppool,
            tc.tile_pool(name="zpsum", bufs=1, space="PSUM") as zpool,
            tc.tile_pool(name="mpool", bufs=12) as mpool,
            tc.tile_pool(name="accp", bufs=2) as accpool,
            tc.tile_pool(name="epool", bufs=3) as epool,
            tc.tile_pool(name="stats", bufs=1) as statpool,
        ):
            xnT_sb = cpool.tile([IN, B], f16)
            nc.sync.dma_start(xnT_sb[:, :], xnT_d[:, :])
            ones_sb = cpool.tile([IN, 1], bf16, tag="ones", name="ones")
            nc.gpsimd.memset(ones_sb[:, :], 1.0)

            w_sb = [wpool.tile([IN, OSHP], f16, tag=f"w{k}", name=f"w{k}")
                    for k in range(K)]
            # two-phase weight upload: the first two chunks' blocks for every
            # k land early so chunk 0/1 matmuls aren't DMA-paced
            for k in range(K):
                nc.sync.dma_start(w_sb[k][:, 0:256],
                                  w_d[:, k * OSHP:k * OSHP + 256])
            for k in range(K):
                nc.sync.dma_start(w_sb[k][:, 256:OSHP],
                                  w_d[:, k * OSHP + 256:(k + 1) * OSHP])

            zsum = zpool.tile([1, B], f32, tag="zsum", name="zsum")
            zout = statpool.tile([1, B], f32, tag="zout", name="zout")

            def mm_plane(c, k):
                ps = ppool.tile([128, B], f32, tag="ps", name=f"ps_{c}_{k}")
                lhsT = w_sb[k][:, c * 128:(c + 1) * 128]
                nc.tensor.matmul(ps[:, 0:512], lhsT, xnT_sb[:, 0:512],
                                 start=True, stop=True)
                nc.tensor.matmul(ps[:, 512:1024], lhsT, xnT_sb[:, 512:1024],
                                 start=True, stop=True)
                return ps

            def emit_exp(c, acc):
                es = epool.tile([128, B], bf16, tag="es", name=f"es_{c}")
                nc.scalar.activation(
                    es[:, :], acc[:, :],
                    mybir.ActivationFunctionType.Exp,
                    bias=0.0, scale=S,
                )
                return es

            def emit_ones_mm(c, es):
                for (c0, cn) in ((0, 512), (512, 512)):
                    nc.tensor.matmul(
                        zsum[0:1, c0:c0 + cn],
                        ones_sb[:, :],
                        es[:, c0:c0 + cn],
                        start=(c == 0), stop=(c == NCHUNK - 1),
                        skip_group_check=True,
                    )

            pend_exp = None    # (chunk, acc) awaiting exp
            pend_ones = []     # [(chunk, escratch)] awaiting ones-matmul
            for c in range(NCHUNK):
                sched = SCHED11 if c % 2 == 0 else SCHED10
                acc = accpool.tile([128, B], f16, tag="acc", name=f"acc_{c}")
                na = 0
                pend_m = []
                for (k, kind) in enumerate(sched):
                    # the previous chunk's ones-matmul rides late in this
                    # chunk's PE stream: its escratch (exp emitted at na==5)
                    # is ready by then, so the PE queue never stalls on it.
                    if k == 11 and pend_ones:
                        emit_ones_mm(*pend_ones.pop(0))
                    ps = mm_plane(c, k)
                    if kind == "a":
                        if na == 0:
                            nc.scalar.copy(acc[:, :], ps[:, :])
                        else:
                            t = mpool.tile([128, B], f16, tag="ts",
                                           name=f"ts_{c}_{na}")
                            nc.scalar.copy(t[:, :], ps[:, :])
                            pend_m.append(t)
                        na += 1
                        # previous chunk's exp rides mid-stream so it never
                        # blocks ACT's queue head at the chunk boundary
                        if na == 5 and pend_exp is not None:
                            pc, pacc = pend_exp
                            pend_ones.append((pc, emit_exp(pc, pacc)))
                            pend_exp = None
                    else:
                        nc.vector.tensor_max(acc[:, :], acc[:, :], ps[:, :])
                        for t in pend_m[:2]:
                            nc.vector.tensor_max(acc[:, :], acc[:, :], t[:, :])
                        pend_m = pend_m[2:]
                for t in pend_m:
                    nc.vector.tensor_max(acc[:, :], acc[:, :], t[:, :])
                pend_exp = (c, acc)

            # drain the tail: exp + ones-matmuls for the last chunks
            pc, pacc = pend_exp
            pend_ones.append((pc, emit_exp(pc, pacc)))
            for item in pend_ones:
                emit_ones_mm(*item)
            nc.vector.tensor_copy(zout[0:1, :], zsum[0:1, :])
            nc.sync.dma_start(out_d[:, :], zout[0:1, :])
    nc.compile()
    return nc


_NC_CACHE = {}


def _get_nc():
    if "v8" not in _NC_CACHE:
        _NC_CACHE["v8"] = _build_nc_v8()
    return _NC_CACHE["v8"]


def _l2norm_np(x, axis):
    n = np.linalg.norm(x, axis=axis, keepdims=True)
    return x / np.maximum(n, 1e-12)


def kernel(input, factor, label, weight):
    from concourse.bass_utils import run_bass_kernel_spmd

    input = np.asarray(input, dtype=np.float32)
    factor = np.asarray(factor, dtype=np.float32)
    label = np.asarray(label)
    weight = np.asarray(weight, dtype=np.float32)

    # ---- host preprocessing ----
    xn = _l2norm_np(input, axis=1)                       # (B, IN) fp32
    wn = _l2norm_np(weight, axis=1)                      # (K, IN, OUT) fp32
    xnT16 = np.ascontiguousarray(xn.T).astype(np.float16)  # (IN, B)

    in_maps = []
    for c in range(NCORES):
        sh = wn[:, :, c * OSH:(c + 1) * OSH]             # (K, IN, OSH)
        shp = np.zeros((K, IN, OSHP), dtype=np.float16)
        shp[:, :, :OSH] = sh.astype(np.float16)          # zero-pad classes
        w_dev = np.ascontiguousarray(
            shp.transpose(1, 0, 2).reshape(IN, K * OSHP)
        )                                                # (IN, K*OSHP), k-major
        in_maps.append({"xnT": xnT16, "w": w_dev})

    nc = _get_nc()
    res = run_bass_kernel_spmd(nc, in_maps, list(range(NCORES)))
    outs = [np.asarray(res.results[c]["out"]) for c in range(NCORES)]  # (1,1024)

    # per-core per-row Z_c[b] = sum_j exp(S*cos_bj) (incl. padded classes:
    # 30 zero-weight columns contribute exp(0)=1 each)
    zsum = np.stack([o.reshape(B) for o in outs]).astype(np.float64)
    Z = zsum.sum(axis=0) - NCORES * (OSHP - OSH)         # remove pad columns

    # ---- host: exact label-column logits ----
    xn16 = xnT16.T.astype(np.float32)                   # device-rounded xn (B, IN)
    wn16 = wn.astype(np.float16).astype(np.float32)     # device-rounded weights
    wl16 = wn16[:, :, label]                            # (K, IN, B)
    v_dev = np.einsum("bf,kfb->kb", xn16, wl16, optimize=True).max(axis=0)  # (B,)
    v16 = v_dev.astype(np.float16).astype(np.float64)   # matches fp16 acc rounding
    wl = wn[:, :, label]                                # (K, IN, B)
    v_true = np.einsum("bf,kfb->kb", xn.astype(np.float32), wl, optimize=True).max(axis=0)

    # margined label logit, replicating the reference formula exactly
    func_a = (np.power(C, factor[:, 0] / 12.0) * MARGIN).astype(np.float32)  # (B,)
    threshold = (math.pi - func_a).astype(np.float32)
    theta = np.arccos(np.clip(v_true, -1.0 + EPS, 1.0 - EPS).astype(np.float32))
    sel = ~(theta > threshold)  # margin applied iff theta <= threshold
    theta_adj = np.where(sel, theta + func_a, theta)
    l_true = (np.cos(theta_adj) * S).astype(np.float64)  # final label logit (B,)

    # ---- host: LSE with label-column replacement (fp64) ----
    Zp = Z - np.exp(S * v16) + np.exp(l_true)
    lse = np.log(Zp)
    loss = np.mean(lse - l_true)

    # ---- host: top-1 accuracy ----
    # Bound the global rowmax from Z: Z <= OUT * exp(S*R) so
    # R >= (ln Z - ln OUT)/S. If both the unmargined and margined label
    # cosines sit clearly below R_low, some other column wins.
    R_low = (np.log(Z) - math.log(OUT)) / S              # (B,) lower bound
    guard = 2e-3
    safe_not_label = (v16 < R_low - guard) & (l_true / S < R_low - guard)
    n_correct = 0
    ambiguous = np.nonzero(~safe_not_label)[0]
    if len(ambiguous) > 0:
        # exact fallback: full-row recompute in fp32 (reference-exact math)
        xa = xn[ambiguous].astype(np.float32)            # (A, IN)
        cos_a = np.einsum("af,kfo->kao", xa, wn.astype(np.float32),
                          optimize=True).max(axis=0)     # (A, OUT)
        th = np.arccos(np.clip(cos_a, -1.0 + EPS, 1.0 - EPS))
        for i, b in enumerate(ambiguous):
            fa = func_a[b]
            row = th[i]
            one = np.zeros(OUT, dtype=bool)
            one[label[b]] = True
            sel_b = one & ~(row > (math.pi - fa))
            logits_b = np.cos(np.where(sel_b, row + fa, row)) * S
            if logits_b.argmax() == label[b]:
                n_correct += 1
    prec1 = n_correct / B * 100.0

    return np.float32(loss), np.float32(prec1)
